# revision 9
# baseline (speedup 1.0000x reference)
"""Two-layer GCN (GCNConv -> ReLU -> GCNConv -> log_softmax) on 8 Trainium2
NeuronCores.

Strategy (graph/data parallel node partitioning):
  * Destination nodes are sharded across the 8 cores and permuted into
    load-balanced 128-node "dst tiles" (host-side, global degree-sorted
    snake over all (core, tile) slots).
  * Phase 1: each core computes g = dinv * (x_shard @ (32*W1)) for its own
    nodes (fp8 DoubleRow matmuls, fp32 PSUM accumulate), stores its g-table
    shard in fp8e4m3 (1024B rows; the 32x scale centers the fp8 range and
    is divided back out in the epilogue). Tables are split in two pieces
    (tiles 0-48 / 49-97) so the AllGather of piece A overlaps phase-1
    compute of piece B.
  * Phase 2: AllGather the fp8 g table pieces.
  * Phase 3: per dst tile, `dma_gather` pulls the source rows for all
    in-edges (edges bucketed by piece/half chunks to fit int16 indices;
    calls split to <=1024 indices; trailing pad slots use negative indices
    so the DGE skips their descriptors); a selection matrix S (fp8, via
    iota/is_equal) turns the per-destination segment-sum into fp8
    DoubleRow PE matmuls accumulated in PSUM.
    Epilogue: out1 = relu(dinv/32*acc + b1); g2 = dinv * (out1 @ 32*W2)
    stored fp8 in 256B rows, piece-major like g.
  * Phase 3.5: AllGather g2 pieces (piece A overlaps phase-3 piece B).
  * Phase 4: same gather + fp8 DoubleRow S-matmul at feature dim 7
    (transposed accumulation), then dinv/32, b2 and log_softmax.

Self-contained: hardcodes shapes; only needs the container toolchain at
/opt/trn_rl_repo.
"""

import sys

for _p in ("/opt/trn_rl_repo",):
    if _p not in sys.path:
        sys.path.insert(0, _p)

import numpy as np
import ml_dtypes

import concourse.bacc as bacc
import concourse.bass as bass
import concourse.tile as tile
from concourse import bass_utils, mybir
from concourse.masks import make_identity

P = 128
FP16 = mybir.dt.float16
FP8 = mybir.dt.float8e4
F32 = mybir.dt.float32
I16 = mybir.dt.int16
I32 = mybir.dt.int32
AX = mybir.AxisListType
ALU = mybir.AluOpType
ACT = mybir.ActivationFunctionType
DR = mybir.MatmulPerfMode.DoubleRow
NP8 = ml_dtypes.float8_e4m3fn

TSCALE = 32.0      # fp8 table scale (power of 2; folded into W host-side)
MAXBLK = 8         # max 128-row blocks per dma_gather call (single packet)


class Cfg:
    def __init__(self, n_nodes=100000, n_cores=8, f_in=1433, f_mid=789, f_out=7,
                 mm_free=512):
        self.n_nodes = n_nodes
        self.n_cores = n_cores
        self.f_in = f_in
        self.kc = (f_in + P - 1) // P          # k-chunks for layer-1 matmul
        assert self.kc % 2 == 0
        self.f_mid = f_mid
        self.fmp8 = ((f_mid + 255) // 256) * 256      # fp8 row bytes: 1024
        self.kc2 = (f_mid + P - 1) // P        # k-chunks for layer-2 matmul
        self.f_out = f_out
        self.g2w = 256                         # fp8 g2 row bytes
        self.ns = n_nodes // n_cores           # nodes per shard (pre-pad)
        assert self.ns * n_cores == n_nodes
        self.t = (self.ns + P - 1) // P        # dst tiles per core
        self.nsp = self.t * P                  # padded shard size
        self.ntot = self.nsp * n_cores         # padded global table rows
        self.ta = (self.t + 1) // 2            # piece-A tiles per core
        self.tb = self.t - self.ta
        self.pa_rows = self.ta * P * n_cores   # piece-A table rows
        self.pb_rows = self.tb * P * n_cores
        self.n_chunks = 4                      # int16 table chunks (2/piece)
        self.vca = (self.pa_rows + 1) // 2     # chunk rows piece A
        self.vcb = (self.pb_rows + 1) // 2
        assert self.vca < 32768 and self.vcb < 32768
        assert self.pa_rows % 2 == 0 and self.pb_rows % 2 == 0
        self.mm_free = mm_free
        # set by preprocess:
        self.kb = None                         # [t][cb] blocks per bucket
        self.bt = None                         # [t] total blocks per tile
        self.btmax = None
        self.kbmax = None


# ----------------------------------------------------------------- host side

def preprocess(x, edge_index, W1, b1, W2, b2, cfg):
    """Shard + permute nodes, bucket edges by (dst tile, src chunk)."""
    N, C = cfg.n_nodes, cfg.n_cores
    src = np.asarray(edge_index[0], dtype=np.int64)
    dst = np.asarray(edge_index[1], dtype=np.int64)
    loop = np.arange(N, dtype=np.int64)
    src = np.concatenate([src, loop])
    dst = np.concatenate([dst, loop])

    deg = np.bincount(dst, minlength=N).astype(np.float64)
    dinv = (1.0 / np.sqrt(deg)).astype(np.float32)

    # ---- global balanced assignment: sort all nodes by in-degree, snake
    # over all (core, tile) slots so every slot's degree sum is near equal.
    indeg = np.bincount(dst, minlength=N)
    nslots = C * cfg.t
    snake = np.concatenate([np.arange(nslots), np.arange(nslots)[::-1]])
    order = np.argsort(-indeg, kind="stable")          # all nodes, deg desc
    slot_seq = np.tile(snake, (N + 2 * nslots - 1) // (2 * nslots))[:N]
    node_slot = np.empty(N, dtype=np.int64)
    node_slot[order] = slot_seq
    node_col = np.empty(N, dtype=np.int64)
    perm = np.argsort(node_slot[order], kind="stable")
    cols = np.empty(N, dtype=np.int64)
    sorted_slots = node_slot[order][perm]
    start = np.searchsorted(sorted_slots, np.arange(nslots))
    cols[perm] = np.arange(N) - start[sorted_slots]
    node_col[order] = cols
    assert node_col.max() < P

    shard_of = node_slot // cfg.t                      # core of each node
    node_tile = node_slot % cfg.t                      # dst tile within core
    # piece-major table row id: piece A = tiles [0, ta), B = [ta, t)
    in_b = node_tile >= cfg.ta
    row_in_piece = np.where(
        in_b,
        shard_of * (cfg.tb * P) + (node_tile - cfg.ta) * P + node_col,
        shard_of * (cfg.ta * P) + node_tile * P + node_col)
    # chunk id 0..3: piece*2 + upper-half
    vc_of = np.where(in_b, cfg.vcb, cfg.vca)
    e_sub = row_in_piece // vc_of
    chunk_of = in_b * 2 + e_sub
    row_in_chunk = row_in_piece - e_sub * vc_of

    nodes_of = []
    for c in range(C):
        nv = np.full(cfg.nsp, -1, dtype=np.int64)
        sel = shard_of == c
        ids = np.nonzero(sel)[0]
        nv[node_tile[ids] * P + node_col[ids]] = ids
        nodes_of.append(nv)

    # ---- bucket edges by (core, dst tile, src chunk)
    e_core = shard_of[dst]
    e_tile = node_tile[dst]
    e_chunk = chunk_of[src]
    e_row = row_in_chunk[src]
    e_dcol = node_col[dst]
    NB = cfg.n_chunks
    counts = np.zeros((C, cfg.t, NB), dtype=np.int64)
    np.add.at(counts, (e_core, e_tile, e_chunk), 1)
    kb = ((counts.max(axis=0) + P - 1) // P).astype(np.int64)   # [t, NB]
    kb = np.maximum(kb, 1)
    cfg.kb = kb
    cfg.bt = kb.sum(axis=1)                   # [t]
    cfg.btmax = int(cfg.bt.max())
    cfg.kbmax = int(kb.max())
    nblk_tot = int(cfg.bt.sum())

    order_all = np.lexsort((e_chunk, e_tile, e_core))
    s_sorted = e_row[order_all].astype(np.int16)
    d_sorted = e_dcol[order_all].astype(np.float16)
    key = (e_core * cfg.t + e_tile)[order_all] * NB + e_chunk[order_all]
    bounds = np.searchsorted(key, np.arange(C * cfg.t * NB + 1))

    # block offsets per (t, cb)
    blk_off = np.zeros((cfg.t, NB), dtype=np.int64)
    run = 0
    for t in range(cfg.t):
        for cb in range(NB):
            blk_off[t, cb] = run
            run += kb[t, cb]

    xpad = np.zeros((cfg.kc * P, N), dtype=np.float32)
    xpad[: cfg.f_in, :] = np.asarray(x, dtype=np.float32).T
    w1h = np.zeros((P, cfg.kc, cfg.f_mid), dtype=np.float32)
    w1t = np.zeros((cfg.kc * P, cfg.f_mid), dtype=np.float32)
    w1t[: cfg.f_in] = np.asarray(W1, dtype=np.float32) * TSCALE
    w1h[:] = w1t.reshape(cfg.kc, P, cfg.f_mid).transpose(1, 0, 2)
    w1h8 = w1h.astype(NP8)
    w2h = np.zeros((P, cfg.kc2, cfg.f_out), dtype=np.float32)
    w2t = np.zeros((cfg.kc2 * P, cfg.f_out), dtype=np.float32)
    w2t[: cfg.f_mid] = np.asarray(W2, dtype=np.float32) * TSCALE
    w2h[:] = w2t.reshape(cfg.kc2, P, cfg.f_out).transpose(1, 0, 2)
    b1r = np.tile(np.asarray(b1, dtype=np.float32)[None, :], (P, 1))
    b2r = np.zeros((P, 8), dtype=np.float32)
    b2r[:, : cfg.f_out] = np.asarray(b2, dtype=np.float32)[None, :]

    in_maps = []
    for c in range(C):
        nv = nodes_of[c]
        valid = nv >= 0
        xs = np.zeros((cfg.kc * P, cfg.nsp), dtype=np.float32)
        xs[:, valid] = xpad[:, nv[valid]]
        # tile-major contiguous layout: [P, t, kc, P]
        xt = np.ascontiguousarray(
            xs.reshape(cfg.kc, P, cfg.t, P).transpose(1, 2, 0, 3)).astype(NP8)
        dvt = np.zeros(cfg.nsp, dtype=np.float32)
        dvt[valid] = dinv[nv[valid]]
        dv = np.ascontiguousarray(dvt.reshape(cfg.t, P).T)
        # idx: per (t, cb): kb*128 int16, idx j at [j%16, off*8 + j//16]
        eidx = np.zeros((P, nblk_tot * 8), dtype=np.int16)
        edst = np.full((P, nblk_tot), 999.0, dtype=np.float16)
        for t in range(cfg.t):
            for cb in range(NB):
                lo = bounds[(c * cfg.t + t) * NB + cb]
                hi = bounds[(c * cfg.t + t) * NB + cb + 1]
                cnt = hi - lo
                nsl = int(kb[t, cb]) * P
                off = int(blk_off[t, cb])
                ai = np.zeros(nsl, dtype=np.int16)
                ai[:cnt] = s_sorted[lo:hi]
                eidx[:, off * 8: off * 8 + nsl // 16] = np.tile(
                    ai.reshape(nsl // 16, 16).T, (8, 1))
                ad = np.full(nsl, 999.0, dtype=np.float16)
                ad[:cnt] = d_sorted[lo:hi]
                edst[:, off: off + int(kb[t, cb])] = ad.reshape(int(kb[t, cb]), P).T
        in_maps.append({
            "xt": xt, "w1": w1h8, "w2": w2h, "b1r": b1r, "b2r": b2r,
            "dinv_t": dv, "eidx": eidx, "edst": edst,
        })
    return in_maps, nodes_of


# --------------------------------------------------------------- device side

def _gather_split(nc, gg, g_src, eit, co, kbb, elem, qnum):
    """Issue dma_gather calls of at most MAXBLK 128-row blocks each."""
    b0 = 0
    while b0 < kbb:
        nb = min(MAXBLK, kbb - b0)
        ni = nb * P
        nc.gpsimd.dma_gather(
            out_ap=gg[:, b0:b0 + nb, :],
            in_ap=g_src,
            idxs_ap=eit[:, (co + b0) * 8: (co + b0) * 8 + ni // 16],
            num_idxs=ni, num_idxs_reg=ni, elem_size=elem,
            single_packet=True, queue_num=qnum)
        b0 += nb


def build(cfg, debug=False):
    nc = bacc.Bacc("TRN2", target_bir_lowering=False, debug=debug,
                   enable_asserts=False, num_devices=cfg.n_cores,
                   num_swdge_queues=4)
    T, NB = cfg.t, cfg.n_chunks
    TA, TB = cfg.ta, cfg.tb
    FM, FMP8, FO, KC, KC2 = cfg.f_mid, cfg.fmp8, cfg.f_out, cfg.kc, cfg.kc2
    G2W = cfg.g2w
    kb, bt, btmax, kbmax = cfg.kb, cfg.bt, cfg.btmax, cfg.kbmax
    nblk_tot = int(bt.sum())
    blk_off = np.zeros((T, NB), dtype=np.int64)
    run = 0
    for t in range(T):
        for cb in range(NB):
            blk_off[t, cb] = run
            run += kb[t, cb]
    tile_off = [int(blk_off[t, 0]) for t in range(T)]

    xt_d = nc.dram_tensor("xt", [P, T, KC, P], FP8, kind="ExternalInput").ap()
    w1_d = nc.dram_tensor("w1", [P, KC, FM], FP8, kind="ExternalInput").ap()
    w2_d = nc.dram_tensor("w2", [P, KC2, FO], F32, kind="ExternalInput").ap()
    b1_d = nc.dram_tensor("b1r", [P, FM], F32, kind="ExternalInput").ap()
    b2_d = nc.dram_tensor("b2r", [P, 8], F32, kind="ExternalInput").ap()
    dv_d = nc.dram_tensor("dinv_t", [P, T], F32, kind="ExternalInput").ap()
    ei_d = nc.dram_tensor("eidx", [P, nblk_tot * 8], I16, kind="ExternalInput").ap()
    ed_d = nc.dram_tensor("edst", [P, nblk_tot], FP16, kind="ExternalInput").ap()
    out_d = nc.dram_tensor("out", [cfg.nsp, FO], F32, kind="ExternalOutput").ap()

    rg = [list(range(cfg.n_cores))]

    with tile.TileContext(nc) as tc:
        with tc.tile_pool(name="res", bufs=1) as res, \
             tc.tile_pool(name="dram", bufs=1, space="DRAM") as dram:
            g_locA = dram.tile([TA * P, FMP8], FP8)
            g_locB = dram.tile([TB * P, FMP8], FP8)
            g_fullA = dram.tile([cfg.pa_rows, FMP8], FP8, addr_space="Shared")
            g_fullB = dram.tile([cfg.pb_rows, FMP8], FP8, addr_space="Shared")
            g2_locA = dram.tile([TA * P, G2W], FP8)
            g2_locB = dram.tile([TB * P, G2W], FP8)
            g2_fullA = dram.tile([cfg.pa_rows, G2W], FP8, addr_space="Shared")
            g2_fullB = dram.tile([cfg.pb_rows, G2W], FP8, addr_space="Shared")
            ss_cache = dram.tile([P, nblk_tot * P], FP8)

            w2_sb = res.tile([P, KC2, FO], F32)
            nc.sync.dma_start(out=w2_sb[:], in_=w2_d[:])
            b1_sb = res.tile([P, FM], F32)
            nc.sync.dma_start(out=b1_sb[:], in_=b1_d[:])
            b2_sb = res.tile([P, 8], F32)
            nc.sync.dma_start(out=b2_sb[:], in_=b2_d[:])
            dv_sb = res.tile([P, T], F32)
            nc.sync.dma_start(out=dv_sb[:], in_=dv_d[:])
            ident = res.tile([P, P], F32)
            make_identity(nc, ident[:])
            iota_i = res.tile([P, P], I32)
            nc.gpsimd.iota(iota_i[:], pattern=[[1, P]], base=0, channel_multiplier=0)
            iota_h = res.tile([P, P], FP16)
            nc.vector.tensor_copy(out=iota_h[:], in_=iota_i[:])

            def g_src_of(cb):
                if cb < 2:
                    return g_fullA[cb * cfg.vca:(cb + 1) * cfg.vca, :]
                return g_fullB[(cb - 2) * cfg.vcb:(cb - 1) * cfg.vcb, :]

            def g2_src_of(cb):
                if cb < 2:
                    return g2_fullA[cb * cfg.vca:(cb + 1) * cfg.vca, :]
                return g2_fullB[(cb - 2) * cfg.vcb:(cb - 1) * cfg.vcb, :]

            # ------------- phase 1: g = dinv * (x @ 32*W1)  (fp8 table, DR)
            with tc.tile_pool(name="p1", bufs=3) as p1, \
                 tc.tile_pool(name="p1w", bufs=1) as p1w, \
                 tc.tile_pool(name="p1ps", bufs=2, space="PSUM") as p1ps:
                w1_sb = p1w.tile([P, KC, FM], FP8)
                nc.sync.dma_start(out=w1_sb[:], in_=w1_d[:])
                for t in range(T):
                    xtile = p1.tile([P, KC, P], FP8, tag="xtile")
                    nc.sync.dma_start(out=xtile[:], in_=xt_d[:, t, :, :])
                    hp = p1ps.tile([P, FM], F32, tag="hp")
                    for f0 in range(0, FM, 256):
                        f1 = min(f0 + 256, FM)
                        for c in range(0, KC, 2):
                            nc.tensor.matmul(
                                out=hp[:, f0:f1], lhsT=xtile[:, c:c + 2, :],
                                rhs=w1_sb[:, c:c + 2, f0:f1],
                                start=(c == 0), stop=(c == KC - 2),
                                perf_mode=DR)
                    gt = p1.tile([P, FMP8], FP8, tag="gt")
                    nc.vector.memset(gt[:, FM:], 0.0)
                    nc.vector.tensor_scalar(
                        out=gt[:, :FM], in0=hp[:], scalar1=dv_sb[:, t:t + 1],
                        scalar2=None, op0=ALU.mult)
                    if t < TA:
                        nc.sync.dma_start(
                            out=g_locA[t * P:(t + 1) * P, :], in_=gt[:])
                    else:
                        nc.sync.dma_start(
                            out=g_locB[(t - TA) * P:(t - TA + 1) * P, :], in_=gt[:])
                    if t == TA - 1:
                        nc.gpsimd.collective_compute(
                            "AllGather", ALU.bypass, replica_groups=rg,
                            ins=[g_locA[:]], outs=[g_fullA[:]])
                # piece-B allgather
                nc.gpsimd.collective_compute(
                    "AllGather", ALU.bypass, replica_groups=rg,
                    ins=[g_locB[:]], outs=[g_fullB[:]])

            # ---------------- phase 3
            with tc.tile_pool(name="p3", bufs=3) as p3, \
                 tc.tile_pool(name="p3g", bufs=5) as p3g, \
                 tc.tile_pool(name="p3acc", bufs=2, space="PSUM") as p3acc, \
                 tc.tile_pool(name="p3ps", bufs=2, space="PSUM") as p3ps:
                for _i in range(5):
                    zz = p3g.tile([P, kbmax, FMP8], FP8, tag="gg")
                    nc.vector.memset(zz[:, :, :], 0.0)
                for t in range(T):
                    btt = int(bt[t])
                    o_t = tile_off[t]
                    eit = p3.tile([P, btmax * 8], I16, tag="eit")
                    nc.sync.dma_start(
                        out=eit[:, : btt * 8],
                        in_=ei_d[:, o_t * 8: (o_t + btt) * 8])
                    edt = p3.tile([P, btmax], FP16, tag="edt")
                    nc.sync.dma_start(out=edt[:, :btt], in_=ed_d[:, o_t: o_t + btt])
                    acc = p3acc.tile([P, FM], F32, tag="acc")
                    for cb in range(NB):
                        kbb = int(kb[t, cb])
                        co = int(blk_off[t, cb]) - o_t
                        gg = p3g.tile([P, kbb, FMP8], FP8, tag="gg",
                                      padded_shape=[P, kbmax, FMP8])
                        _gather_split(nc, gg, g_src_of(cb), eit, co, kbb,
                                      FMP8, cb)
                        ss = p3.tile([P, kbb, P], FP8, tag="ss",
                                     padded_shape=[P, kbmax, P])
                        nc.vector.tensor_tensor(
                            out=ss[:, :, :],
                            in0=edt[:, co: co + kbb].unsqueeze(2)
                                .to_broadcast([P, kbb, P]),
                            in1=iota_h[:].unsqueeze(1).to_broadcast([P, kbb, P]),
                            op=ALU.is_equal)
                        nc.sync.dma_start(
                            out=ss_cache[:, (o_t + co) * P:(o_t + co + kbb) * P],
                            in_=ss[:, :, :])
                        # fp8 DoubleRow: two 128-edge blocks per matmul
                        b = 0
                        while b < kbb:
                            first = (co + b == 0)
                            if b + 2 <= kbb:
                                last = (co + b + 2 == btt)
                                for f0 in range(0, FM, 256):
                                    f1 = min(f0 + 256, FM)
                                    nc.tensor.matmul(
                                        out=acc[:, f0:f1],
                                        lhsT=ss[:, b:b + 2, :],
                                        rhs=gg[:, b:b + 2, f0:f1],
                                        start=first, stop=last,
                                        perf_mode=DR)
                                b += 2
                            else:
                                last = (co + b + 1 == btt)
                                for f0 in range(0, FM, cfg.mm_free):
                                    f1 = min(f0 + cfg.mm_free, FM)
                                    nc.tensor.matmul(
                                        out=acc[:, f0:f1], lhsT=ss[:, b, :],
                                        rhs=gg[:, b, f0:f1],
                                        start=first, stop=last)
                                b += 1
                    # epilogue: out1 = relu(dinv/32*acc + b1)
                    t1 = p3.tile([P, FM], F32, tag="t1")
                    nc.vector.tensor_scalar(
                        out=t1[:], in0=acc[:], scalar1=dv_sb[:, t:t + 1],
                        scalar2=1.0 / TSCALE, op0=ALU.mult, op1=ALU.mult)
                    nc.vector.tensor_add(out=t1[:], in0=t1[:], in1=b1_sb[:])
                    nc.vector.tensor_scalar_max(out=t1[:], in0=t1[:], scalar1=0.0)
                    # g2T = (32*W2)^T @ t1^T
                    g2t = p3ps.tile([P, P], F32, tag="g2t")
                    for c in range(KC2):
                        f0 = c * P
                        cw = min(P, FM - f0)
                        tp = p3ps.tile([P, P], F32, tag="tp")
                        nc.tensor.transpose(
                            out=tp[:cw, :], in_=t1[:, f0:f0 + cw], identity=ident[:])
                        tps = p3.tile([P, P], F32, tag="tps")
                        nc.vector.tensor_copy(out=tps[:cw, :], in_=tp[:cw, :])
                        nc.tensor.matmul(
                            out=g2t[:FO, :], lhsT=w2_sb[:cw, c, :], rhs=tps[:cw, :],
                            start=(c == 0), stop=(c == KC2 - 1))
                    drp = p3ps.tile([P, P], F32, tag="tp")
                    nc.tensor.transpose(
                        out=drp[:], in_=dv_sb[:, t:t + 1].to_broadcast([P, P]),
                        identity=ident[:])
                    dr = p3.tile([P, P], F32, tag="dr")
                    nc.vector.tensor_copy(out=dr[:], in_=drp[:])
                    g2s = p3.tile([P, P], F32, tag="g2s")
                    nc.vector.tensor_tensor(
                        out=g2s[:FO, :], in0=g2t[:FO, :], in1=dr[:FO, :], op=ALU.mult)
                    g2ntp = p3ps.tile([P, 8], F32, tag="tp")
                    nc.tensor.transpose(
                        out=g2ntp[:, :FO], in_=g2s[:FO, :], identity=ident[:FO, :FO])
                    g2o = p3.tile([P, G2W], FP8, tag="g2o")
                    nc.vector.memset(g2o[:], 0.0)
                    nc.vector.tensor_copy(out=g2o[:, :FO], in_=g2ntp[:, :FO])
                    if t < TA:
                        nc.sync.dma_start(
                            out=g2_locA[t * P:(t + 1) * P, :], in_=g2o[:])
                    else:
                        nc.sync.dma_start(
                            out=g2_locB[(t - TA) * P:(t - TA + 1) * P, :],
                            in_=g2o[:])
                    if t == TA - 1:
                        nc.gpsimd.collective_compute(
                            "AllGather", ALU.bypass, replica_groups=rg,
                            ins=[g2_locA[:]], outs=[g2_fullA[:]])
                nc.gpsimd.collective_compute(
                    "AllGather", ALU.bypass, replica_groups=rg,
                    ins=[g2_locB[:]], outs=[g2_fullB[:]])

            # ---------------- phase 4
            with tc.tile_pool(name="p4", bufs=3) as p4, \
                 tc.tile_pool(name="p4g", bufs=5) as p4g, \
                 tc.tile_pool(name="p4ps", bufs=2, space="PSUM") as p4ps:
                for _i in range(5):
                    zz = p4g.tile([P, kbmax, G2W], FP8, tag="gg2")
                    nc.vector.memset(zz[:, :, :], 0.0)
                for t in range(T):
                    btt = int(bt[t])
                    o_t = tile_off[t]
                    eit = p4.tile([P, btmax * 8], I16, tag="eit4")
                    nc.sync.dma_start(
                        out=eit[:, : btt * 8],
                        in_=ei_d[:, o_t * 8: (o_t + btt) * 8])
                    acc2 = p4ps.tile([P, P], F32, tag="acc2")
                    for cb in range(NB):
                        kbb = int(kb[t, cb])
                        co = int(blk_off[t, cb]) - o_t
                        gg2 = p4g.tile([P, kbb, G2W], FP8, tag="gg2",
                                       padded_shape=[P, kbmax, G2W])
                        _gather_split(nc, gg2, g2_src_of(cb), eit, co, kbb,
                                      G2W, cb)
                        ss = p4.tile([P, kbb, P], FP8, tag="ss4",
                                     padded_shape=[P, kbmax, P])
                        nc.sync.dma_start(
                            out=ss[:, :, :],
                            in_=ss_cache[:, (o_t + co) * P:(o_t + co + kbb) * P])
                        b = 0
                        while b < kbb:
                            first = (co + b == 0)
                            if b + 2 <= kbb:
                                last = (co + b + 2 == btt)
                                nc.tensor.matmul(
                                    out=acc2[:8, :],
                                    lhsT=gg2[:, b:b + 2, :8],
                                    rhs=ss[:, b:b + 2, :],
                                    start=first, stop=last, perf_mode=DR)
                                b += 2
                            else:
                                last = (co + b + 1 == btt)
                                nc.tensor.matmul(
                                    out=acc2[:8, :], lhsT=gg2[:, b, :8],
                                    rhs=ss[:, b, :],
                                    start=first, stop=last)
                                b += 1
                    t2s = p4.tile([P, P], F32, tag="t2s")
                    nc.vector.tensor_copy(out=t2s[:8, :], in_=acc2[:8, :])
                    t2ntp = p4ps.tile([P, 8], F32, tag="t2ntp")
                    nc.tensor.transpose(
                        out=t2ntp[:, :8], in_=t2s[:8, :], identity=ident[:8, :8])
                    tf = p4.tile([P, 8], F32, tag="tf")
                    nc.vector.tensor_scalar(
                        out=tf[:], in0=t2ntp[:], scalar1=dv_sb[:, t:t + 1],
                        scalar2=1.0 / TSCALE, op0=ALU.mult, op1=ALU.mult)
                    nc.vector.tensor_add(out=tf[:], in0=tf[:], in1=b2_sb[:])
                    nm = p4.tile([P, 1], F32, tag="nm")
                    nc.vector.tensor_reduce(
                        out=nm[:], in_=tf[:, :FO], axis=AX.X, op=ALU.max, negate=True)
                    ex = p4.tile([P, 8], F32, tag="ex")
                    se = p4.tile([P, 1], F32, tag="se")
                    nc.scalar.activation(
                        out=ex[:, :FO], in_=tf[:, :FO], func=ACT.Exp,
                        bias=nm[:, :1], scale=1.0, accum_out=se[:, :1])
                    lse = p4.tile([P, 1], F32, tag="lse")
                    nc.scalar.activation(out=lse[:], in_=se[:], func=ACT.Ln)
                    of = p4.tile([P, 8], F32, tag="of")
                    nc.vector.tensor_scalar(
                        out=of[:, :FO], in0=tf[:, :FO], scalar1=nm[:, :1],
                        scalar2=lse[:, :1], op0=ALU.add, op1=ALU.subtract)
                    nc.sync.dma_start(out=out_d[t * P:(t + 1) * P, :], in_=of[:, :FO])

    nc.compile()
    return nc


# ------------------------------------------------------------------ runner

def _run(inputs, cfg=None, trace=False, trace_kwargs=None):
    cfg = cfg or Cfg()
    in_maps, nodes_of = preprocess(
        inputs["x"], inputs["edge_index"], inputs["W1"], inputs["b1"],
        inputs["W2"], inputs["b2"], cfg)
    nc = build(cfg)
    res = bass_utils.run_bass_kernel_spmd(
        nc, in_maps, core_ids=list(range(cfg.n_cores)), trace=trace,
        **(trace_kwargs or {}))
    out = np.zeros((cfg.n_nodes, cfg.f_out), dtype=np.float32)
    for c in range(cfg.n_cores):
        oc = res.results[c]["out"]
        nv = nodes_of[c]
        valid = nv >= 0
        out[nv[valid]] = oc[valid]
    return out, res


def kernel(**inputs):
    out, _ = _run(inputs)
    return out


# revision 13
# speedup vs baseline: 1.1281x; 1.1281x over previous
"""Two-layer GCN (GCNConv -> ReLU -> GCNConv -> log_softmax) on 8 Trainium2
NeuronCores.

Strategy (graph/data parallel node partitioning):
  * Destination nodes are sharded across the 8 cores and permuted into
    load-balanced 128-node "dst tiles" (host-side, global degree-sorted
    snake over all (core, tile) slots).
  * Phase 1: each core computes g = dinv * (x_shard @ (32*W1)) for its own
    nodes (fp8 DoubleRow matmuls, fp32 PSUM accumulate), stores its g-table
    shard in fp8e4m3 (1024B rows; the 32x scale centers the fp8 range and
    is divided back out in the epilogue). Tables are split in two pieces
    (tiles 0-48 / 49-97) so the AllGather of piece A overlaps phase-1
    compute of piece B.
  * Phase 2: AllGather the fp8 g table pieces.
  * Phase 3: per dst tile, `dma_gather` pulls the source rows for all
    in-edges (edges bucketed by piece/half chunks to fit int16 indices;
    calls split to <=1024 indices; trailing pad slots use negative indices
    so the DGE skips their descriptors); a selection matrix S (fp8, via
    iota/is_equal) turns the per-destination segment-sum into fp8
    DoubleRow PE matmuls accumulated in PSUM.
    Epilogue: out1 = relu(dinv/32*acc + b1); g2 = dinv * (out1 @ 32*W2)
    stored fp8 in 256B rows, piece-major like g.
  * Phase 3.5: AllGather g2 pieces (piece A overlaps phase-3 piece B).
  * Phase 4: same gather + fp8 DoubleRow S-matmul at feature dim 7
    (transposed accumulation), then dinv/32, b2 and log_softmax.

Self-contained: hardcodes shapes; only needs the container toolchain at
/opt/trn_rl_repo.
"""

import sys

for _p in ("/opt/trn_rl_repo",):
    if _p not in sys.path:
        sys.path.insert(0, _p)

import numpy as np
import ml_dtypes

import concourse.bacc as bacc
import concourse.bass as bass
import concourse.tile as tile
from concourse import bass_utils, mybir
from concourse.masks import make_identity

P = 128
FP16 = mybir.dt.float16
FP8 = mybir.dt.float8e4
F32 = mybir.dt.float32
I16 = mybir.dt.int16
I32 = mybir.dt.int32
AX = mybir.AxisListType
ALU = mybir.AluOpType
ACT = mybir.ActivationFunctionType
DR = mybir.MatmulPerfMode.DoubleRow
NP8 = ml_dtypes.float8_e4m3fn

TSCALE = 32.0      # fp8 table scale (power of 2; folded into W host-side)
MAXBLK = 8         # max 128-row blocks per dma_gather call (single packet)


class Cfg:
    def __init__(self, n_nodes=100000, n_cores=8, f_in=1433, f_mid=789, f_out=7,
                 mm_free=512):
        self.n_nodes = n_nodes
        self.n_cores = n_cores
        self.f_in = f_in
        self.kc = (f_in + P - 1) // P          # k-chunks for layer-1 matmul
        assert self.kc % 2 == 0
        self.f_mid = f_mid
        self.fmp8 = ((f_mid + 255) // 256) * 256      # fp8 row bytes: 1024
        self.kc2 = (f_mid + P - 1) // P        # k-chunks for layer-2 matmul
        self.f_out = f_out
        self.g2w = 256                         # fp8 g2 row bytes
        self.ns = n_nodes // n_cores           # nodes per shard (pre-pad)
        assert self.ns * n_cores == n_nodes
        self.t = (self.ns + P - 1) // P        # dst tiles per core
        self.nsp = self.t * P                  # padded shard size
        self.ntot = self.nsp * n_cores         # padded global table rows
        self.n_chunks = 4                      # pieces == int16 chunks
        q, r = divmod(self.t, 4)
        self.pt = [q + (i < r) for i in range(4)]      # tiles per piece
        self.pt0 = [sum(self.pt[:i]) for i in range(5)]  # piece tile starts
        self.prows = [p * P * n_cores for p in self.pt]  # table rows/piece
        assert all(v < 32768 for v in self.prows)
        self.mm_free = mm_free
        # set by preprocess:
        self.kb = None                         # [t][cb] blocks per bucket
        self.bt = None                         # [t] total blocks per tile
        self.btmax = None
        self.kbmax = None


# ----------------------------------------------------------------- host side

def preprocess(x, edge_index, W1, b1, W2, b2, cfg):
    """Shard + permute nodes, bucket edges by (dst tile, src chunk)."""
    N, C = cfg.n_nodes, cfg.n_cores
    src = np.asarray(edge_index[0], dtype=np.int64)
    dst = np.asarray(edge_index[1], dtype=np.int64)
    loop = np.arange(N, dtype=np.int64)
    src = np.concatenate([src, loop])
    dst = np.concatenate([dst, loop])

    deg = np.bincount(dst, minlength=N).astype(np.float64)
    dinv = (1.0 / np.sqrt(deg)).astype(np.float32)

    # ---- global balanced assignment: sort all nodes by in-degree, snake
    # over all (core, tile) slots so every slot's degree sum is near equal.
    indeg = np.bincount(dst, minlength=N)
    nslots = C * cfg.t
    snake = np.concatenate([np.arange(nslots), np.arange(nslots)[::-1]])
    order = np.argsort(-indeg, kind="stable")          # all nodes, deg desc
    slot_seq = np.tile(snake, (N + 2 * nslots - 1) // (2 * nslots))[:N]
    node_slot = np.empty(N, dtype=np.int64)
    node_slot[order] = slot_seq
    node_col = np.empty(N, dtype=np.int64)
    perm = np.argsort(node_slot[order], kind="stable")
    cols = np.empty(N, dtype=np.int64)
    sorted_slots = node_slot[order][perm]
    start = np.searchsorted(sorted_slots, np.arange(nslots))
    cols[perm] = np.arange(N) - start[sorted_slots]
    node_col[order] = cols
    assert node_col.max() < P

    shard_of = node_slot // cfg.t                      # core of each node
    node_tile = node_slot % cfg.t                      # dst tile within core
    # piece-major table row id; chunk == piece (4 pieces of ~t/4 tiles)
    pt0 = np.array(cfg.pt0[:4])
    piece = np.searchsorted(np.array(cfg.pt0[1:5]), node_tile, side="right")
    ptiles = np.array(cfg.pt)[piece]
    row_in_chunk = (shard_of * ptiles * P
                    + (node_tile - pt0[piece]) * P + node_col)
    chunk_of = piece

    nodes_of = []
    for c in range(C):
        nv = np.full(cfg.nsp, -1, dtype=np.int64)
        sel = shard_of == c
        ids = np.nonzero(sel)[0]
        nv[node_tile[ids] * P + node_col[ids]] = ids
        nodes_of.append(nv)

    # ---- bucket edges by (core, dst tile, src chunk)
    e_core = shard_of[dst]
    e_tile = node_tile[dst]
    e_chunk = chunk_of[src]
    e_row = row_in_chunk[src]
    e_dcol = node_col[dst]
    NB = cfg.n_chunks
    counts = np.zeros((C, cfg.t, NB), dtype=np.int64)
    np.add.at(counts, (e_core, e_tile, e_chunk), 1)
    kb = ((counts.max(axis=0) + P - 1) // P).astype(np.int64)   # [t, NB]
    kb = np.maximum(kb, 1)
    cfg.kb = kb
    cfg.bt = kb.sum(axis=1)                   # [t]
    cfg.btmax = int(cfg.bt.max())
    cfg.kbmax = int(kb.max())
    nblk_tot = int(cfg.bt.sum())

    order_all = np.lexsort((e_chunk, e_tile, e_core))
    s_sorted = e_row[order_all].astype(np.int16)
    d_sorted = e_dcol[order_all].astype(np.float16)
    key = (e_core * cfg.t + e_tile)[order_all] * NB + e_chunk[order_all]
    bounds = np.searchsorted(key, np.arange(C * cfg.t * NB + 1))

    # block offsets per (t, cb)
    blk_off = np.zeros((cfg.t, NB), dtype=np.int64)
    run = 0
    for t in range(cfg.t):
        for cb in range(NB):
            blk_off[t, cb] = run
            run += kb[t, cb]

    xpad = np.zeros((cfg.kc * P, N), dtype=np.float32)
    xpad[: cfg.f_in, :] = np.asarray(x, dtype=np.float32).T
    w1h = np.zeros((P, cfg.kc, cfg.f_mid), dtype=np.float32)
    w1t = np.zeros((cfg.kc * P, cfg.f_mid), dtype=np.float32)
    w1t[: cfg.f_in] = np.asarray(W1, dtype=np.float32) * TSCALE
    w1h[:] = w1t.reshape(cfg.kc, P, cfg.f_mid).transpose(1, 0, 2)
    w1h8 = w1h.astype(NP8)
    w2h = np.zeros((P, cfg.kc2, cfg.f_out), dtype=np.float32)
    w2t = np.zeros((cfg.kc2 * P, cfg.f_out), dtype=np.float32)
    w2t[: cfg.f_mid] = np.asarray(W2, dtype=np.float32) * TSCALE
    w2h[:] = w2t.reshape(cfg.kc2, P, cfg.f_out).transpose(1, 0, 2)
    b1r = np.tile(np.asarray(b1, dtype=np.float32)[None, :], (P, 1))
    b2r = np.zeros((P, 8), dtype=np.float32)
    b2r[:, : cfg.f_out] = np.asarray(b2, dtype=np.float32)[None, :]

    in_maps = []
    for c in range(C):
        nv = nodes_of[c]
        valid = nv >= 0
        xs = np.zeros((cfg.kc * P, cfg.nsp), dtype=np.float32)
        xs[:, valid] = xpad[:, nv[valid]]
        # tile-major contiguous layout: [P, t, kc, P]
        xt = np.ascontiguousarray(
            xs.reshape(cfg.kc, P, cfg.t, P).transpose(1, 2, 0, 3)).astype(NP8)
        dvt = np.zeros(cfg.nsp, dtype=np.float32)
        dvt[valid] = dinv[nv[valid]]
        dv = np.ascontiguousarray(dvt.reshape(cfg.t, P).T)
        # idx: per (t, cb): kb*128 int16, idx j at [j%16, off*8 + j//16]
        eidx = np.zeros((P, nblk_tot * 8), dtype=np.int16)
        edst = np.full((P, nblk_tot), 999.0, dtype=np.float16)
        for t in range(cfg.t):
            for cb in range(NB):
                lo = bounds[(c * cfg.t + t) * NB + cb]
                hi = bounds[(c * cfg.t + t) * NB + cb + 1]
                cnt = hi - lo
                nsl = int(kb[t, cb]) * P
                off = int(blk_off[t, cb])
                ai = np.zeros(nsl, dtype=np.int16)
                ai[:cnt] = s_sorted[lo:hi]
                eidx[:, off * 8: off * 8 + nsl // 16] = np.tile(
                    ai.reshape(nsl // 16, 16).T, (8, 1))
                ad = np.full(nsl, 999.0, dtype=np.float16)
                ad[:cnt] = d_sorted[lo:hi]
                edst[:, off: off + int(kb[t, cb])] = ad.reshape(int(kb[t, cb]), P).T
        in_maps.append({
            "xt": xt, "w1": w1h8, "w2": w2h, "b1r": b1r, "b2r": b2r,
            "dinv_t": dv, "eidx": eidx, "edst": edst,
        })
    return in_maps, nodes_of


# --------------------------------------------------------------- device side

def _gather_split(nc, gg, g_src, eit, co, kbb, elem, qnum):
    """Issue dma_gather calls of at most MAXBLK 128-row blocks each."""
    b0 = 0
    while b0 < kbb:
        nb = min(MAXBLK, kbb - b0)
        ni = nb * P
        nc.gpsimd.dma_gather(
            out_ap=gg[:, b0:b0 + nb, :],
            in_ap=g_src,
            idxs_ap=eit[:, (co + b0) * 8: (co + b0) * 8 + ni // 16],
            num_idxs=ni, num_idxs_reg=ni, elem_size=elem,
            single_packet=True, queue_num=qnum)
        b0 += nb


def build(cfg, debug=False):
    nc = bacc.Bacc("TRN2", target_bir_lowering=False, debug=debug,
                   enable_asserts=False, num_devices=cfg.n_cores,
                   num_swdge_queues=4)
    T, NB = cfg.t, cfg.n_chunks
    FM, FMP8, FO, KC, KC2 = cfg.f_mid, cfg.fmp8, cfg.f_out, cfg.kc, cfg.kc2
    G2W = cfg.g2w
    kb, bt, btmax, kbmax = cfg.kb, cfg.bt, cfg.btmax, cfg.kbmax
    nblk_tot = int(bt.sum())
    blk_off = np.zeros((T, NB), dtype=np.int64)
    run = 0
    for t in range(T):
        for cb in range(NB):
            blk_off[t, cb] = run
            run += kb[t, cb]
    tile_off = [int(blk_off[t, 0]) for t in range(T)]

    xt_d = nc.dram_tensor("xt", [P, T, KC, P], FP8, kind="ExternalInput").ap()
    w1_d = nc.dram_tensor("w1", [P, KC, FM], FP8, kind="ExternalInput").ap()
    w2_d = nc.dram_tensor("w2", [P, KC2, FO], F32, kind="ExternalInput").ap()
    b1_d = nc.dram_tensor("b1r", [P, FM], F32, kind="ExternalInput").ap()
    b2_d = nc.dram_tensor("b2r", [P, 8], F32, kind="ExternalInput").ap()
    dv_d = nc.dram_tensor("dinv_t", [P, T], F32, kind="ExternalInput").ap()
    ei_d = nc.dram_tensor("eidx", [P, nblk_tot * 8], I16, kind="ExternalInput").ap()
    ed_d = nc.dram_tensor("edst", [P, nblk_tot], FP16, kind="ExternalInput").ap()
    out_d = nc.dram_tensor("out", [cfg.nsp, FO], F32, kind="ExternalOutput").ap()

    rg = [list(range(cfg.n_cores))]

    with tile.TileContext(nc) as tc:
        with tc.tile_pool(name="res", bufs=1) as res, \
             tc.tile_pool(name="dram", bufs=1, space="DRAM") as dram:
            g_loc = [dram.tile([cfg.pt[i] * P, FMP8], FP8, name=f"gloc{i}")
                     for i in range(4)]
            g_full = [dram.tile([cfg.prows[i], FMP8], FP8, addr_space="Shared",
                                name=f"gfull{i}") for i in range(4)]
            g2_loc = [dram.tile([cfg.pt[i] * P, G2W], FP8, name=f"g2loc{i}")
                      for i in range(4)]
            g2_full = [dram.tile([cfg.prows[i], G2W], FP8, addr_space="Shared",
                                 name=f"g2full{i}") for i in range(4)]

            w2_sb = res.tile([P, KC2, FO], F32)
            nc.sync.dma_start(out=w2_sb[:], in_=w2_d[:])
            b1_sb = res.tile([P, FM], F32)
            nc.sync.dma_start(out=b1_sb[:], in_=b1_d[:])
            b2_sb = res.tile([P, 8], F32)
            nc.sync.dma_start(out=b2_sb[:], in_=b2_d[:])
            dv_sb = res.tile([P, T], F32)
            nc.sync.dma_start(out=dv_sb[:], in_=dv_d[:])
            ident = res.tile([P, P], F32)
            make_identity(nc, ident[:])
            iota_i = res.tile([P, P], I32)
            nc.gpsimd.iota(iota_i[:], pattern=[[1, P]], base=0, channel_multiplier=0)
            iota_h = res.tile([P, P], FP16)
            nc.vector.tensor_copy(out=iota_h[:], in_=iota_i[:])

            def g_src_of(cb):
                return g_full[cb][:]

            def g2_src_of(cb):
                return g2_full[cb][:]

            # ------------- phase 1: g = dinv * (x @ 32*W1)  (fp8 table, DR)
            with tc.tile_pool(name="p1", bufs=3) as p1, \
                 tc.tile_pool(name="p1w", bufs=1) as p1w, \
                 tc.tile_pool(name="p1ps", bufs=2, space="PSUM") as p1ps:
                w1_sb = p1w.tile([P, KC, FM], FP8)
                nc.sync.dma_start(out=w1_sb[:], in_=w1_d[:])
                for t in range(T):
                    xtile = p1.tile([P, KC, P], FP8, tag="xtile")
                    nc.sync.dma_start(out=xtile[:], in_=xt_d[:, t, :, :])
                    hp = p1ps.tile([P, FM], F32, tag="hp")
                    for f0 in range(0, FM, 256):
                        f1 = min(f0 + 256, FM)
                        for c in range(0, KC, 2):
                            nc.tensor.matmul(
                                out=hp[:, f0:f1], lhsT=xtile[:, c:c + 2, :],
                                rhs=w1_sb[:, c:c + 2, f0:f1],
                                start=(c == 0), stop=(c == KC - 2),
                                perf_mode=DR)
                    gt = p1.tile([P, FMP8], FP8, tag="gt")
                    nc.vector.memset(gt[:, FM:], 0.0)
                    nc.vector.tensor_scalar(
                        out=gt[:, :FM], in0=hp[:], scalar1=dv_sb[:, t:t + 1],
                        scalar2=None, op0=ALU.mult)
                    pc = next(i for i in range(4) if t < cfg.pt0[i + 1])
                    tl = t - cfg.pt0[pc]
                    nc.sync.dma_start(
                        out=g_loc[pc][tl * P:(tl + 1) * P, :], in_=gt[:])
                    if t == cfg.pt0[pc + 1] - 1:
                        nc.gpsimd.collective_compute(
                            "AllGather", ALU.bypass, replica_groups=rg,
                            ins=[g_loc[pc][:]], outs=[g_full[pc][:]])

            # ---------------- phase 3
            with tc.tile_pool(name="p3", bufs=3) as p3, \
                 tc.tile_pool(name="p3g", bufs=4) as p3g, \
                 tc.tile_pool(name="p3acc", bufs=2, space="PSUM") as p3acc, \
                 tc.tile_pool(name="p3ps", bufs=2, space="PSUM") as p3ps:
                for _i in range(4):
                    zz = p3g.tile([P, kbmax, FMP8], FP8, tag="gg")
                    nc.vector.memset(zz[:, :, :], 0.0)
                for t in range(T):
                    btt = int(bt[t])
                    o_t = tile_off[t]
                    eit = p3.tile([P, btmax * 8], I16, tag="eit")
                    nc.sync.dma_start(
                        out=eit[:, : btt * 8],
                        in_=ei_d[:, o_t * 8: (o_t + btt) * 8])
                    edt = p3.tile([P, btmax], FP16, tag="edt")
                    nc.sync.dma_start(out=edt[:, :btt], in_=ed_d[:, o_t: o_t + btt])
                    acc = p3acc.tile([P, FM], F32, tag="acc")
                    for cb in range(NB):
                        kbb = int(kb[t, cb])
                        co = int(blk_off[t, cb]) - o_t
                        gg = p3g.tile([P, kbb, FMP8], FP8, tag="gg",
                                      padded_shape=[P, kbmax, FMP8])
                        _gather_split(nc, gg, g_src_of(cb), eit, co, kbb,
                                      FMP8, cb)
                        ss = p3.tile([P, kbb, P], FP8, tag="ss",
                                     padded_shape=[P, kbmax, P])
                        nc.vector.tensor_tensor(
                            out=ss[:, :, :],
                            in0=edt[:, co: co + kbb].unsqueeze(2)
                                .to_broadcast([P, kbb, P]),
                            in1=iota_h[:].unsqueeze(1).to_broadcast([P, kbb, P]),
                            op=ALU.is_equal)
                        # fp8 DoubleRow: two 128-edge blocks per matmul
                        b = 0
                        while b < kbb:
                            first = (co + b == 0)
                            if b + 2 <= kbb:
                                last = (co + b + 2 == btt)
                                for f0 in range(0, FM, 256):
                                    f1 = min(f0 + 256, FM)
                                    nc.tensor.matmul(
                                        out=acc[:, f0:f1],
                                        lhsT=ss[:, b:b + 2, :],
                                        rhs=gg[:, b:b + 2, f0:f1],
                                        start=first, stop=last,
                                        perf_mode=DR)
                                b += 2
                            else:
                                last = (co + b + 1 == btt)
                                for f0 in range(0, FM, cfg.mm_free):
                                    f1 = min(f0 + cfg.mm_free, FM)
                                    nc.tensor.matmul(
                                        out=acc[:, f0:f1], lhsT=ss[:, b, :],
                                        rhs=gg[:, b, f0:f1],
                                        start=first, stop=last)
                                b += 1
                    # epilogue: out1 = relu(dinv/32*acc + b1)
                    t1 = p3.tile([P, FM], F32, tag="t1")
                    nc.vector.tensor_scalar(
                        out=t1[:], in0=acc[:], scalar1=dv_sb[:, t:t + 1],
                        scalar2=1.0 / TSCALE, op0=ALU.mult, op1=ALU.mult)
                    nc.vector.tensor_add(out=t1[:], in0=t1[:], in1=b1_sb[:])
                    nc.vector.tensor_scalar_max(out=t1[:], in0=t1[:], scalar1=0.0)
                    # g2T = (32*W2)^T @ t1^T
                    g2t = p3ps.tile([P, P], F32, tag="g2t")
                    for c in range(KC2):
                        f0 = c * P
                        cw = min(P, FM - f0)
                        tp = p3ps.tile([P, P], F32, tag="tp")
                        nc.tensor.transpose(
                            out=tp[:cw, :], in_=t1[:, f0:f0 + cw], identity=ident[:])
                        tps = p3.tile([P, P], F32, tag="tps")
                        nc.vector.tensor_copy(out=tps[:cw, :], in_=tp[:cw, :])
                        nc.tensor.matmul(
                            out=g2t[:FO, :], lhsT=w2_sb[:cw, c, :], rhs=tps[:cw, :],
                            start=(c == 0), stop=(c == KC2 - 1))
                    drp = p3ps.tile([P, P], F32, tag="tp")
                    nc.tensor.transpose(
                        out=drp[:], in_=dv_sb[:, t:t + 1].to_broadcast([P, P]),
                        identity=ident[:])
                    dr = p3.tile([P, P], F32, tag="dr")
                    nc.vector.tensor_copy(out=dr[:], in_=drp[:])
                    g2s = p3.tile([P, P], F32, tag="g2s")
                    nc.vector.tensor_tensor(
                        out=g2s[:FO, :], in0=g2t[:FO, :], in1=dr[:FO, :], op=ALU.mult)
                    g2ntp = p3ps.tile([P, 8], F32, tag="tp")
                    nc.tensor.transpose(
                        out=g2ntp[:, :FO], in_=g2s[:FO, :], identity=ident[:FO, :FO])
                    g2o = p3.tile([P, G2W], FP8, tag="g2o")
                    nc.vector.memset(g2o[:], 0.0)
                    nc.vector.tensor_copy(out=g2o[:, :FO], in_=g2ntp[:, :FO])
                    pc = next(i for i in range(4) if t < cfg.pt0[i + 1])
                    tl = t - cfg.pt0[pc]
                    nc.sync.dma_start(
                        out=g2_loc[pc][tl * P:(tl + 1) * P, :], in_=g2o[:])
                    if t == cfg.pt0[pc + 1] - 1:
                        nc.gpsimd.collective_compute(
                            "AllGather", ALU.bypass, replica_groups=rg,
                            ins=[g2_loc[pc][:]], outs=[g2_full[pc][:]])

            # ---------------- phase 4
            with tc.tile_pool(name="p4", bufs=3) as p4, \
                 tc.tile_pool(name="p4g", bufs=4) as p4g, \
                 tc.tile_pool(name="p4ps", bufs=2, space="PSUM") as p4ps:
                for _i in range(4):
                    zz = p4g.tile([P, kbmax, G2W], FP8, tag="gg2")
                    nc.vector.memset(zz[:, :, :], 0.0)
                for t in range(T):
                    btt = int(bt[t])
                    o_t = tile_off[t]
                    eit = p4.tile([P, btmax * 8], I16, tag="eit4")
                    nc.sync.dma_start(
                        out=eit[:, : btt * 8],
                        in_=ei_d[:, o_t * 8: (o_t + btt) * 8])
                    edt = p4.tile([P, btmax], FP16, tag="edt4")
                    nc.sync.dma_start(out=edt[:, :btt], in_=ed_d[:, o_t: o_t + btt])
                    acc2 = p4ps.tile([P, P], F32, tag="acc2")
                    for cb in range(NB):
                        kbb = int(kb[t, cb])
                        co = int(blk_off[t, cb]) - o_t
                        gg2 = p4g.tile([P, kbb, G2W], FP8, tag="gg2",
                                       padded_shape=[P, kbmax, G2W])
                        _gather_split(nc, gg2, g2_src_of(cb), eit, co, kbb,
                                      G2W, cb)
                        ss = p4.tile([P, kbb, P], FP8, tag="ss4",
                                     padded_shape=[P, kbmax, P])
                        nc.vector.tensor_tensor(
                            out=ss[:, :, :],
                            in0=edt[:, co: co + kbb].unsqueeze(2)
                                .to_broadcast([P, kbb, P]),
                            in1=iota_h[:].unsqueeze(1).to_broadcast([P, kbb, P]),
                            op=ALU.is_equal)
                        b = 0
                        while b < kbb:
                            first = (co + b == 0)
                            if b + 2 <= kbb:
                                last = (co + b + 2 == btt)
                                nc.tensor.matmul(
                                    out=acc2[:8, :],
                                    lhsT=gg2[:, b:b + 2, :8],
                                    rhs=ss[:, b:b + 2, :],
                                    start=first, stop=last, perf_mode=DR)
                                b += 2
                            else:
                                last = (co + b + 1 == btt)
                                nc.tensor.matmul(
                                    out=acc2[:8, :], lhsT=gg2[:, b, :8],
                                    rhs=ss[:, b, :],
                                    start=first, stop=last)
                                b += 1
                    t2s = p4.tile([P, P], F32, tag="t2s")
                    nc.vector.tensor_copy(out=t2s[:8, :], in_=acc2[:8, :])
                    t2ntp = p4ps.tile([P, 8], F32, tag="t2ntp")
                    nc.tensor.transpose(
                        out=t2ntp[:, :8], in_=t2s[:8, :], identity=ident[:8, :8])
                    tf = p4.tile([P, 8], F32, tag="tf")
                    nc.vector.tensor_scalar(
                        out=tf[:], in0=t2ntp[:], scalar1=dv_sb[:, t:t + 1],
                        scalar2=1.0 / TSCALE, op0=ALU.mult, op1=ALU.mult)
                    nc.vector.tensor_add(out=tf[:], in0=tf[:], in1=b2_sb[:])
                    nm = p4.tile([P, 1], F32, tag="nm")
                    nc.vector.tensor_reduce(
                        out=nm[:], in_=tf[:, :FO], axis=AX.X, op=ALU.max, negate=True)
                    ex = p4.tile([P, 8], F32, tag="ex")
                    se = p4.tile([P, 1], F32, tag="se")
                    nc.scalar.activation(
                        out=ex[:, :FO], in_=tf[:, :FO], func=ACT.Exp,
                        bias=nm[:, :1], scale=1.0, accum_out=se[:, :1])
                    lse = p4.tile([P, 1], F32, tag="lse")
                    nc.scalar.activation(out=lse[:], in_=se[:], func=ACT.Ln)
                    of = p4.tile([P, 8], F32, tag="of")
                    nc.vector.tensor_scalar(
                        out=of[:, :FO], in0=tf[:, :FO], scalar1=nm[:, :1],
                        scalar2=lse[:, :1], op0=ALU.add, op1=ALU.subtract)
                    nc.sync.dma_start(out=out_d[t * P:(t + 1) * P, :], in_=of[:, :FO])

    nc.compile()
    return nc


# ------------------------------------------------------------------ runner

def _run(inputs, cfg=None, trace=False, trace_kwargs=None):
    cfg = cfg or Cfg()
    in_maps, nodes_of = preprocess(
        inputs["x"], inputs["edge_index"], inputs["W1"], inputs["b1"],
        inputs["W2"], inputs["b2"], cfg)
    nc = build(cfg)
    res = bass_utils.run_bass_kernel_spmd(
        nc, in_maps, core_ids=list(range(cfg.n_cores)), trace=trace,
        **(trace_kwargs or {}))
    out = np.zeros((cfg.n_nodes, cfg.f_out), dtype=np.float32)
    for c in range(cfg.n_cores):
        oc = res.results[c]["out"]
        nv = nodes_of[c]
        valid = nv >= 0
        out[nv[valid]] = oc[valid]
    return out, res


def kernel(**inputs):
    out, _ = _run(inputs)
    return out


# revision 14
# speedup vs baseline: 1.1313x; 1.0028x over previous
"""Two-layer GCN (GCNConv -> ReLU -> GCNConv -> log_softmax) on 8 Trainium2
NeuronCores.

Strategy (graph/data parallel node partitioning):
  * Destination nodes are sharded across the 8 cores and permuted into
    load-balanced 128-node "dst tiles" (host-side, global degree-sorted
    snake over all (core, tile) slots).
  * Phase 1: each core computes g = dinv * (x_shard @ (32*W1)) for its own
    nodes (fp8 DoubleRow matmuls, fp32 PSUM accumulate), stores its g-table
    shard in fp8e4m3 (1024B rows; the 32x scale centers the fp8 range and
    is divided back out in the epilogue). Tables are split in two pieces
    (tiles 0-48 / 49-97) so the AllGather of piece A overlaps phase-1
    compute of piece B.
  * Phase 2: AllGather the fp8 g table pieces.
  * Phase 3: per dst tile, `dma_gather` pulls the source rows for all
    in-edges (edges bucketed by piece/half chunks to fit int16 indices;
    calls split to <=1024 indices; trailing pad slots use negative indices
    so the DGE skips their descriptors); a selection matrix S (fp8, via
    iota/is_equal) turns the per-destination segment-sum into fp8
    DoubleRow PE matmuls accumulated in PSUM.
    Epilogue: out1 = relu(dinv/32*acc + b1); g2 = dinv * (out1 @ 32*W2)
    stored fp8 in 256B rows, piece-major like g.
  * Phase 3.5: AllGather g2 pieces (piece A overlaps phase-3 piece B).
  * Phase 4: same gather + fp8 DoubleRow S-matmul at feature dim 7
    (transposed accumulation), then dinv/32, b2 and log_softmax.

Self-contained: hardcodes shapes; only needs the container toolchain at
/opt/trn_rl_repo.
"""

import sys

for _p in ("/opt/trn_rl_repo",):
    if _p not in sys.path:
        sys.path.insert(0, _p)

import numpy as np
import ml_dtypes

import concourse.bacc as bacc
import concourse.bass as bass
import concourse.tile as tile
from concourse import bass_utils, mybir
from concourse.masks import make_identity

P = 128
FP16 = mybir.dt.float16
FP8 = mybir.dt.float8e4
F32 = mybir.dt.float32
I16 = mybir.dt.int16
I32 = mybir.dt.int32
AX = mybir.AxisListType
ALU = mybir.AluOpType
ACT = mybir.ActivationFunctionType
DR = mybir.MatmulPerfMode.DoubleRow
NP8 = ml_dtypes.float8_e4m3fn

TSCALE = 32.0      # fp8 table scale (power of 2; folded into W host-side)
MAXBLK = 8         # max 128-row blocks per dma_gather call (single packet)


class Cfg:
    def __init__(self, n_nodes=100000, n_cores=8, f_in=1433, f_mid=789, f_out=7,
                 mm_free=512):
        self.n_nodes = n_nodes
        self.n_cores = n_cores
        self.f_in = f_in
        self.kc = (f_in + P - 1) // P          # k-chunks for layer-1 matmul
        assert self.kc % 2 == 0
        self.f_mid = f_mid
        self.fmp8 = ((f_mid + 255) // 256) * 256      # fp8 row bytes: 1024
        self.kc2 = (f_mid + P - 1) // P        # k-chunks for layer-2 matmul
        self.f_out = f_out
        self.g2w = 256                         # fp8 g2 row bytes
        self.ns = n_nodes // n_cores           # nodes per shard (pre-pad)
        assert self.ns * n_cores == n_nodes
        self.t = (self.ns + P - 1) // P        # dst tiles per core
        self.nsp = self.t * P                  # padded shard size
        self.ntot = self.nsp * n_cores         # padded global table rows
        self.n_chunks = 4                      # pieces == int16 chunks
        q, r = divmod(self.t, 4)
        self.pt = [q + (i < r) for i in range(4)]      # tiles per piece
        self.pt0 = [sum(self.pt[:i]) for i in range(5)]  # piece tile starts
        self.prows = [p * P * n_cores for p in self.pt]  # table rows/piece
        assert all(v < 32768 for v in self.prows)
        self.mm_free = mm_free
        # set by preprocess:
        self.kb = None                         # [t][cb] blocks per bucket
        self.bt = None                         # [t] total blocks per tile
        self.btmax = None
        self.kbmax = None


# ----------------------------------------------------------------- host side

def preprocess(x, edge_index, W1, b1, W2, b2, cfg):
    """Shard + permute nodes, bucket edges by (dst tile, src chunk)."""
    N, C = cfg.n_nodes, cfg.n_cores
    src = np.asarray(edge_index[0], dtype=np.int64)
    dst = np.asarray(edge_index[1], dtype=np.int64)
    loop = np.arange(N, dtype=np.int64)
    src = np.concatenate([src, loop])
    dst = np.concatenate([dst, loop])

    deg = np.bincount(dst, minlength=N).astype(np.float64)
    dinv = (1.0 / np.sqrt(deg)).astype(np.float32)

    # ---- global balanced assignment: sort all nodes by in-degree, snake
    # over all (core, tile) slots so every slot's degree sum is near equal.
    indeg = np.bincount(dst, minlength=N)
    nslots = C * cfg.t
    snake = np.concatenate([np.arange(nslots), np.arange(nslots)[::-1]])
    order = np.argsort(-indeg, kind="stable")          # all nodes, deg desc
    slot_seq = np.tile(snake, (N + 2 * nslots - 1) // (2 * nslots))[:N]
    node_slot = np.empty(N, dtype=np.int64)
    node_slot[order] = slot_seq
    node_col = np.empty(N, dtype=np.int64)
    perm = np.argsort(node_slot[order], kind="stable")
    cols = np.empty(N, dtype=np.int64)
    sorted_slots = node_slot[order][perm]
    start = np.searchsorted(sorted_slots, np.arange(nslots))
    cols[perm] = np.arange(N) - start[sorted_slots]
    node_col[order] = cols
    assert node_col.max() < P

    shard_of = node_slot // cfg.t                      # core of each node
    node_tile = node_slot % cfg.t                      # dst tile within core
    # piece-major table row id; chunk == piece (4 pieces of ~t/4 tiles)
    pt0 = np.array(cfg.pt0[:4])
    piece = np.searchsorted(np.array(cfg.pt0[1:5]), node_tile, side="right")
    ptiles = np.array(cfg.pt)[piece]
    row_in_chunk = (shard_of * ptiles * P
                    + (node_tile - pt0[piece]) * P + node_col)
    chunk_of = piece

    nodes_of = []
    for c in range(C):
        nv = np.full(cfg.nsp, -1, dtype=np.int64)
        sel = shard_of == c
        ids = np.nonzero(sel)[0]
        nv[node_tile[ids] * P + node_col[ids]] = ids
        nodes_of.append(nv)

    # ---- bucket edges by (core, dst tile, src chunk)
    e_core = shard_of[dst]
    e_tile = node_tile[dst]
    e_chunk = chunk_of[src]
    e_row = row_in_chunk[src]
    e_dcol = node_col[dst]
    NB = cfg.n_chunks
    counts = np.zeros((C, cfg.t, NB), dtype=np.int64)
    np.add.at(counts, (e_core, e_tile, e_chunk), 1)
    kb = ((counts.max(axis=0) + P - 1) // P).astype(np.int64)   # [t, NB]
    kb = np.maximum(kb, 1)
    cfg.kb = kb
    cfg.bt = kb.sum(axis=1)                   # [t]
    cfg.btmax = int(cfg.bt.max())
    cfg.kbmax = int(kb.max())
    nblk_tot = int(cfg.bt.sum())

    order_all = np.lexsort((e_chunk, e_tile, e_core))
    s_sorted = e_row[order_all].astype(np.int16)
    d_sorted = e_dcol[order_all].astype(np.float16)
    key = (e_core * cfg.t + e_tile)[order_all] * NB + e_chunk[order_all]
    bounds = np.searchsorted(key, np.arange(C * cfg.t * NB + 1))

    # block offsets per (t, cb)
    blk_off = np.zeros((cfg.t, NB), dtype=np.int64)
    run = 0
    for t in range(cfg.t):
        for cb in range(NB):
            blk_off[t, cb] = run
            run += kb[t, cb]

    xpad = np.zeros((cfg.kc * P, N), dtype=np.float32)
    xpad[: cfg.f_in, :] = np.asarray(x, dtype=np.float32).T
    w1h = np.zeros((P, cfg.kc, cfg.f_mid), dtype=np.float32)
    w1t = np.zeros((cfg.kc * P, cfg.f_mid), dtype=np.float32)
    w1t[: cfg.f_in] = np.asarray(W1, dtype=np.float32) * TSCALE
    w1h[:] = w1t.reshape(cfg.kc, P, cfg.f_mid).transpose(1, 0, 2)
    w1h8 = w1h.astype(NP8)
    w2h = np.zeros((P, cfg.kc2, cfg.f_out), dtype=np.float32)
    w2t = np.zeros((cfg.kc2 * P, cfg.f_out), dtype=np.float32)
    w2t[: cfg.f_mid] = np.asarray(W2, dtype=np.float32) * TSCALE
    w2h[:] = w2t.reshape(cfg.kc2, P, cfg.f_out).transpose(1, 0, 2)
    b1r = np.tile(np.asarray(b1, dtype=np.float32)[None, :], (P, 1))
    b2r = np.zeros((P, 8), dtype=np.float32)
    b2r[:, : cfg.f_out] = np.asarray(b2, dtype=np.float32)[None, :]

    in_maps = []
    for c in range(C):
        nv = nodes_of[c]
        valid = nv >= 0
        xs = np.zeros((cfg.kc * P, cfg.nsp), dtype=np.float32)
        xs[:, valid] = xpad[:, nv[valid]]
        # tile-major contiguous layout: [P, t, kc, P]
        xt = np.ascontiguousarray(
            xs.reshape(cfg.kc, P, cfg.t, P).transpose(1, 2, 0, 3)).astype(NP8)
        dvt = np.zeros(cfg.nsp, dtype=np.float32)
        dvt[valid] = dinv[nv[valid]]
        dv = np.ascontiguousarray(dvt.reshape(cfg.t, P).T)
        dv32 = np.ascontiguousarray(dv / TSCALE)
        # idx: per (t, cb): kb*128 int16, idx j at [j%16, off*8 + j//16]
        eidx = np.zeros((P, nblk_tot * 8), dtype=np.int16)
        edst = np.full((P, nblk_tot), 999.0, dtype=np.float16)
        for t in range(cfg.t):
            for cb in range(NB):
                lo = bounds[(c * cfg.t + t) * NB + cb]
                hi = bounds[(c * cfg.t + t) * NB + cb + 1]
                cnt = hi - lo
                nsl = int(kb[t, cb]) * P
                off = int(blk_off[t, cb])
                ai = np.zeros(nsl, dtype=np.int16)
                ai[:cnt] = s_sorted[lo:hi]
                eidx[:, off * 8: off * 8 + nsl // 16] = np.tile(
                    ai.reshape(nsl // 16, 16).T, (8, 1))
                ad = np.full(nsl, 999.0, dtype=np.float16)
                ad[:cnt] = d_sorted[lo:hi]
                edst[:, off: off + int(kb[t, cb])] = ad.reshape(int(kb[t, cb]), P).T
        in_maps.append({
            "xt": xt, "w1": w1h8, "w2": w2h, "b1r": b1r, "b2r": b2r,
            "dinv_t": dv, "dinv32_t": dv32, "eidx": eidx, "edst": edst,
        })
    return in_maps, nodes_of


# --------------------------------------------------------------- device side

def _gather_split(nc, gg, g_src, eit, co, kbb, elem, qnum):
    """Issue dma_gather calls of at most MAXBLK 128-row blocks each."""
    b0 = 0
    while b0 < kbb:
        nb = min(MAXBLK, kbb - b0)
        ni = nb * P
        nc.gpsimd.dma_gather(
            out_ap=gg[:, b0:b0 + nb, :],
            in_ap=g_src,
            idxs_ap=eit[:, (co + b0) * 8: (co + b0) * 8 + ni // 16],
            num_idxs=ni, num_idxs_reg=ni, elem_size=elem,
            single_packet=True, queue_num=qnum)
        b0 += nb


def build(cfg, debug=False):
    nc = bacc.Bacc("TRN2", target_bir_lowering=False, debug=debug,
                   enable_asserts=False, num_devices=cfg.n_cores,
                   num_swdge_queues=4)
    T, NB = cfg.t, cfg.n_chunks
    FM, FMP8, FO, KC, KC2 = cfg.f_mid, cfg.fmp8, cfg.f_out, cfg.kc, cfg.kc2
    G2W = cfg.g2w
    kb, bt, btmax, kbmax = cfg.kb, cfg.bt, cfg.btmax, cfg.kbmax
    nblk_tot = int(bt.sum())
    blk_off = np.zeros((T, NB), dtype=np.int64)
    run = 0
    for t in range(T):
        for cb in range(NB):
            blk_off[t, cb] = run
            run += kb[t, cb]
    tile_off = [int(blk_off[t, 0]) for t in range(T)]

    xt_d = nc.dram_tensor("xt", [P, T, KC, P], FP8, kind="ExternalInput").ap()
    w1_d = nc.dram_tensor("w1", [P, KC, FM], FP8, kind="ExternalInput").ap()
    w2_d = nc.dram_tensor("w2", [P, KC2, FO], F32, kind="ExternalInput").ap()
    b1_d = nc.dram_tensor("b1r", [P, FM], F32, kind="ExternalInput").ap()
    b2_d = nc.dram_tensor("b2r", [P, 8], F32, kind="ExternalInput").ap()
    dv_d = nc.dram_tensor("dinv_t", [P, T], F32, kind="ExternalInput").ap()
    dv32_d = nc.dram_tensor("dinv32_t", [P, T], F32, kind="ExternalInput").ap()
    ei_d = nc.dram_tensor("eidx", [P, nblk_tot * 8], I16, kind="ExternalInput").ap()
    ed_d = nc.dram_tensor("edst", [P, nblk_tot], FP16, kind="ExternalInput").ap()
    out_d = nc.dram_tensor("out", [cfg.nsp, FO], F32, kind="ExternalOutput").ap()

    rg = [list(range(cfg.n_cores))]

    with tile.TileContext(nc) as tc:
        with tc.tile_pool(name="res", bufs=1) as res, \
             tc.tile_pool(name="dram", bufs=1, space="DRAM") as dram:
            g_loc = [dram.tile([cfg.pt[i] * P, FMP8], FP8, name=f"gloc{i}")
                     for i in range(4)]
            g_full = [dram.tile([cfg.prows[i], FMP8], FP8, addr_space="Shared",
                                name=f"gfull{i}") for i in range(4)]
            g2_loc = [dram.tile([cfg.pt[i] * P, G2W], FP8, name=f"g2loc{i}")
                      for i in range(4)]
            g2_full = [dram.tile([cfg.prows[i], G2W], FP8, addr_space="Shared",
                                 name=f"g2full{i}") for i in range(4)]

            w2_sb = res.tile([P, KC2, FO], F32)
            nc.sync.dma_start(out=w2_sb[:], in_=w2_d[:])
            b1_sb = res.tile([P, FM], F32)
            nc.sync.dma_start(out=b1_sb[:], in_=b1_d[:])
            b2_sb = res.tile([P, 8], F32)
            nc.sync.dma_start(out=b2_sb[:], in_=b2_d[:])
            dv_sb = res.tile([P, T], F32)
            nc.sync.dma_start(out=dv_sb[:], in_=dv_d[:])
            dv32_sb = res.tile([P, T], F32)
            nc.sync.dma_start(out=dv32_sb[:], in_=dv32_d[:])
            ident = res.tile([P, P], F32)
            make_identity(nc, ident[:])
            iota_i = res.tile([P, P], I32)
            nc.gpsimd.iota(iota_i[:], pattern=[[1, P]], base=0, channel_multiplier=0)
            iota_h = res.tile([P, P], FP16)
            nc.vector.tensor_copy(out=iota_h[:], in_=iota_i[:])

            def g_src_of(cb):
                return g_full[cb][:]

            def g2_src_of(cb):
                return g2_full[cb][:]

            # ------------- phase 1: g = dinv * (x @ 32*W1)  (fp8 table, DR)
            with tc.tile_pool(name="p1", bufs=4) as p1, \
                 tc.tile_pool(name="p1w", bufs=1) as p1w, \
                 tc.tile_pool(name="p1ps", bufs=3, space="PSUM") as p1ps:
                w1_sb = p1w.tile([P, KC, FM], FP8)
                nc.sync.dma_start(out=w1_sb[:], in_=w1_d[:])
                for t in range(T):
                    xtile = p1.tile([P, KC, P], FP8, tag="xtile")
                    nc.sync.dma_start(out=xtile[:], in_=xt_d[:, t, :, :])
                    hp = p1ps.tile([P, FM], F32, tag="hp")
                    for f0 in range(0, FM, 256):
                        f1 = min(f0 + 256, FM)
                        for c in range(0, KC, 2):
                            nc.tensor.matmul(
                                out=hp[:, f0:f1], lhsT=xtile[:, c:c + 2, :],
                                rhs=w1_sb[:, c:c + 2, f0:f1],
                                start=(c == 0), stop=(c == KC - 2),
                                perf_mode=DR)
                    gt = p1.tile([P, FMP8], FP8, tag="gt")
                    nc.vector.memset(gt[:, FM:], 0.0)
                    nc.vector.tensor_scalar(
                        out=gt[:, :FM], in0=hp[:], scalar1=dv_sb[:, t:t + 1],
                        scalar2=None, op0=ALU.mult)
                    pc = next(i for i in range(4) if t < cfg.pt0[i + 1])
                    tl = t - cfg.pt0[pc]
                    nc.sync.dma_start(
                        out=g_loc[pc][tl * P:(tl + 1) * P, :], in_=gt[:])
                    if t == cfg.pt0[pc + 1] - 1:
                        nc.gpsimd.collective_compute(
                            "AllGather", ALU.bypass, replica_groups=rg,
                            ins=[g_loc[pc][:]], outs=[g_full[pc][:]])

            # ---------------- phase 3
            with tc.tile_pool(name="p3", bufs=3) as p3, \
                 tc.tile_pool(name="p3g", bufs=4) as p3g, \
                 tc.tile_pool(name="p3acc", bufs=2, space="PSUM") as p3acc, \
                 tc.tile_pool(name="p3ps", bufs=2, space="PSUM") as p3ps:
                for _i in range(4):
                    zz = p3g.tile([P, kbmax, FMP8], FP8, tag="gg")
                    nc.vector.memset(zz[:, :, :], 0.0)
                for t in range(T):
                    btt = int(bt[t])
                    o_t = tile_off[t]
                    eit = p3.tile([P, btmax * 8], I16, tag="eit")
                    nc.sync.dma_start(
                        out=eit[:, : btt * 8],
                        in_=ei_d[:, o_t * 8: (o_t + btt) * 8])
                    edt = p3.tile([P, btmax], FP16, tag="edt")
                    nc.sync.dma_start(out=edt[:, :btt], in_=ed_d[:, o_t: o_t + btt])
                    acc = p3acc.tile([P, FM], F32, tag="acc")
                    for cb in range(NB):
                        kbb = int(kb[t, cb])
                        co = int(blk_off[t, cb]) - o_t
                        gg = p3g.tile([P, kbb, FMP8], FP8, tag="gg",
                                      padded_shape=[P, kbmax, FMP8])
                        _gather_split(nc, gg, g_src_of(cb), eit, co, kbb,
                                      FMP8, cb)
                        ss = p3.tile([P, kbb, P], FP8, tag="ss",
                                     padded_shape=[P, kbmax, P])
                        nc.vector.tensor_tensor(
                            out=ss[:, :, :],
                            in0=edt[:, co: co + kbb].unsqueeze(2)
                                .to_broadcast([P, kbb, P]),
                            in1=iota_h[:].unsqueeze(1).to_broadcast([P, kbb, P]),
                            op=ALU.is_equal)
                        # fp8 DoubleRow: two 128-edge blocks per matmul
                        b = 0
                        while b < kbb:
                            first = (co + b == 0)
                            if b + 2 <= kbb:
                                last = (co + b + 2 == btt)
                                for f0 in range(0, FM, 256):
                                    f1 = min(f0 + 256, FM)
                                    nc.tensor.matmul(
                                        out=acc[:, f0:f1],
                                        lhsT=ss[:, b:b + 2, :],
                                        rhs=gg[:, b:b + 2, f0:f1],
                                        start=first, stop=last,
                                        perf_mode=DR)
                                b += 2
                            else:
                                last = (co + b + 1 == btt)
                                for f0 in range(0, FM, cfg.mm_free):
                                    f1 = min(f0 + cfg.mm_free, FM)
                                    nc.tensor.matmul(
                                        out=acc[:, f0:f1], lhsT=ss[:, b, :],
                                        rhs=gg[:, b, f0:f1],
                                        start=first, stop=last)
                                b += 1
                    # epilogue: out1 = relu(dinv/32*acc + b1)
                    t1 = p3.tile([P, FM], F32, tag="t1")
                    nc.vector.scalar_tensor_tensor(
                        out=t1[:], in0=acc[:], scalar=dv32_sb[:, t:t + 1],
                        in1=b1_sb[:], op0=ALU.mult, op1=ALU.add)
                    nc.vector.tensor_scalar_max(out=t1[:], in0=t1[:], scalar1=0.0)
                    # g2T = (32*W2)^T @ t1^T
                    g2t = p3ps.tile([P, P], F32, tag="g2t")
                    for c in range(KC2):
                        f0 = c * P
                        cw = min(P, FM - f0)
                        tp = p3ps.tile([P, P], F32, tag="tp")
                        nc.tensor.transpose(
                            out=tp[:cw, :], in_=t1[:, f0:f0 + cw], identity=ident[:])
                        tps = p3.tile([P, P], F32, tag="tps")
                        nc.vector.tensor_copy(out=tps[:cw, :], in_=tp[:cw, :])
                        nc.tensor.matmul(
                            out=g2t[:FO, :], lhsT=w2_sb[:cw, c, :], rhs=tps[:cw, :],
                            start=(c == 0), stop=(c == KC2 - 1))
                    drp = p3ps.tile([P, P], F32, tag="tp")
                    nc.tensor.transpose(
                        out=drp[:], in_=dv_sb[:, t:t + 1].to_broadcast([P, P]),
                        identity=ident[:])
                    dr = p3.tile([P, P], F32, tag="dr")
                    nc.vector.tensor_copy(out=dr[:], in_=drp[:])
                    g2s = p3.tile([P, P], F32, tag="g2s")
                    nc.vector.tensor_tensor(
                        out=g2s[:FO, :], in0=g2t[:FO, :], in1=dr[:FO, :], op=ALU.mult)
                    g2ntp = p3ps.tile([P, 8], F32, tag="tp")
                    nc.tensor.transpose(
                        out=g2ntp[:, :FO], in_=g2s[:FO, :], identity=ident[:FO, :FO])
                    g2o = p3.tile([P, G2W], FP8, tag="g2o")
                    nc.vector.memset(g2o[:], 0.0)
                    nc.vector.tensor_copy(out=g2o[:, :FO], in_=g2ntp[:, :FO])
                    pc = next(i for i in range(4) if t < cfg.pt0[i + 1])
                    tl = t - cfg.pt0[pc]
                    nc.sync.dma_start(
                        out=g2_loc[pc][tl * P:(tl + 1) * P, :], in_=g2o[:])
                    for qc in range(4):
                        if t == min(cfg.pt0[qc + 1] + 2, T - 1):
                            nc.gpsimd.collective_compute(
                                "AllGather", ALU.bypass, replica_groups=rg,
                                ins=[g2_loc[qc][:]], outs=[g2_full[qc][:]])

            # ---------------- phase 4
            with tc.tile_pool(name="p4", bufs=3) as p4, \
                 tc.tile_pool(name="p4g", bufs=4) as p4g, \
                 tc.tile_pool(name="p4ps", bufs=2, space="PSUM") as p4ps:
                for _i in range(4):
                    zz = p4g.tile([P, kbmax, G2W], FP8, tag="gg2")
                    nc.vector.memset(zz[:, :, :], 0.0)
                for t in range(T):
                    btt = int(bt[t])
                    o_t = tile_off[t]
                    eit = p4.tile([P, btmax * 8], I16, tag="eit4")
                    nc.sync.dma_start(
                        out=eit[:, : btt * 8],
                        in_=ei_d[:, o_t * 8: (o_t + btt) * 8])
                    edt = p4.tile([P, btmax], FP16, tag="edt4")
                    nc.sync.dma_start(out=edt[:, :btt], in_=ed_d[:, o_t: o_t + btt])
                    acc2 = p4ps.tile([P, P], F32, tag="acc2")
                    for cb in range(NB):
                        kbb = int(kb[t, cb])
                        co = int(blk_off[t, cb]) - o_t
                        gg2 = p4g.tile([P, kbb, G2W], FP8, tag="gg2",
                                       padded_shape=[P, kbmax, G2W])
                        _gather_split(nc, gg2, g2_src_of(cb), eit, co, kbb,
                                      G2W, cb)
                        ss = p4.tile([P, kbb, P], FP8, tag="ss4",
                                     padded_shape=[P, kbmax, P])
                        nc.vector.tensor_tensor(
                            out=ss[:, :, :],
                            in0=edt[:, co: co + kbb].unsqueeze(2)
                                .to_broadcast([P, kbb, P]),
                            in1=iota_h[:].unsqueeze(1).to_broadcast([P, kbb, P]),
                            op=ALU.is_equal)
                        b = 0
                        while b < kbb:
                            first = (co + b == 0)
                            if b + 2 <= kbb:
                                last = (co + b + 2 == btt)
                                nc.tensor.matmul(
                                    out=acc2[:8, :],
                                    lhsT=gg2[:, b:b + 2, :8],
                                    rhs=ss[:, b:b + 2, :],
                                    start=first, stop=last, perf_mode=DR)
                                b += 2
                            else:
                                last = (co + b + 1 == btt)
                                nc.tensor.matmul(
                                    out=acc2[:8, :], lhsT=gg2[:, b, :8],
                                    rhs=ss[:, b, :],
                                    start=first, stop=last)
                                b += 1
                    t2s = p4.tile([P, P], F32, tag="t2s")
                    nc.vector.tensor_copy(out=t2s[:8, :], in_=acc2[:8, :])
                    t2ntp = p4ps.tile([P, 8], F32, tag="t2ntp")
                    nc.tensor.transpose(
                        out=t2ntp[:, :8], in_=t2s[:8, :], identity=ident[:8, :8])
                    tf = p4.tile([P, 8], F32, tag="tf")
                    nc.vector.scalar_tensor_tensor(
                        out=tf[:], in0=t2ntp[:], scalar=dv32_sb[:, t:t + 1],
                        in1=b2_sb[:], op0=ALU.mult, op1=ALU.add)
                    nm = p4.tile([P, 1], F32, tag="nm")
                    nc.vector.tensor_reduce(
                        out=nm[:], in_=tf[:, :FO], axis=AX.X, op=ALU.max, negate=True)
                    ex = p4.tile([P, 8], F32, tag="ex")
                    se = p4.tile([P, 1], F32, tag="se")
                    nc.scalar.activation(
                        out=ex[:, :FO], in_=tf[:, :FO], func=ACT.Exp,
                        bias=nm[:, :1], scale=1.0, accum_out=se[:, :1])
                    lse = p4.tile([P, 1], F32, tag="lse")
                    nc.scalar.activation(out=lse[:], in_=se[:], func=ACT.Ln)
                    of = p4.tile([P, 8], F32, tag="of")
                    nc.vector.tensor_scalar(
                        out=of[:, :FO], in0=tf[:, :FO], scalar1=nm[:, :1],
                        scalar2=lse[:, :1], op0=ALU.add, op1=ALU.subtract)
                    nc.sync.dma_start(out=out_d[t * P:(t + 1) * P, :], in_=of[:, :FO])

    nc.compile()
    return nc


# ------------------------------------------------------------------ runner

def _run(inputs, cfg=None, trace=False, trace_kwargs=None):
    cfg = cfg or Cfg()
    in_maps, nodes_of = preprocess(
        inputs["x"], inputs["edge_index"], inputs["W1"], inputs["b1"],
        inputs["W2"], inputs["b2"], cfg)
    nc = build(cfg)
    res = bass_utils.run_bass_kernel_spmd(
        nc, in_maps, core_ids=list(range(cfg.n_cores)), trace=trace,
        **(trace_kwargs or {}))
    out = np.zeros((cfg.n_nodes, cfg.f_out), dtype=np.float32)
    for c in range(cfg.n_cores):
        oc = res.results[c]["out"]
        nv = nodes_of[c]
        valid = nv >= 0
        out[nv[valid]] = oc[valid]
    return out, res


def kernel(**inputs):
    out, _ = _run(inputs)
    return out
